# revision 3
# baseline (speedup 1.0000x reference)
"""Trainium2 Bass kernel for nn_DepthRenderer (superquadric depth renderer), v2.

Sharding: rows round-robin over 8 cores (core c owns image rows r = 8*lr+c,
lr=0..44).  Per-core layout [128 lanes, 45 lrows, 5 xblocks]; lane = x%128,
xblock = x//128.  Each core renders all 8 SQs (constants baked as immediates
into one SPMD program) and min-accumulates depth on device; host concatenates.

Sparsity: per-SQ bounding rects on a coarse host grid (see _host_rects).
Rect columns of all live SQs are packed into one flat column axis of size
STOT; per-SQ geometry writes packed planes, the sample/occupancy pipeline
runs batched per chunk of SQs, software-pipelined across chunks so the
Vector (DVE) and Scalar (ACT) engines overlap.

v2 speedups over the baseline:
  - bf16 on the sample-dim DVE ops (PL build, |PL|, S/f adds, weights):
    tensor_scalar/scalar_tensor_tensor run in 4x DVE mode on packed bf16.
  - fp32 tensor_tensor ops replaced by scalar_tensor_tensor with a dummy
    x1.0 scalar stage: InstTensorScalarPtr supports the 2x_2p DVE mode in
    fp32 (plain InstTensorTensor does not).
  - ACT Abs eliminated (|PL| via one DVE STT max(-x,x)); ACT Identity(+1)
    eliminated (occ denominator built with a DVE tensor_scalar add).
  - per-SQ constant folds: K0 = exp(-TAU*occ0) folded into HG and the
    boundary dt terms (so the occ0 cumsum add disappears); the F clamp is
    applied in log domain (one TS: min(e1*lnf, ln 1.085)).
  - batched ACT chain per SQ-chunk (ln/lnS/lnf/F/E shared-scale ops cover a
    whole chunk in one instruction).

Math notes (exact rewrites of the reference, up to fp rounding):
  - a == sizes  =>  X = |loc|/a + eps = |pts_loc| + eps  (sizes cancel)
  - samples lie on the ray: tau_s = d1/nu2 + hcl*t_s, so boundary segment
    lengths need no norms: dt0 = |tau_0|*||d||, dt10 = |1.5 - tau_9|*||d||
  - occ = sigmoid(SHARP*(1-F)) = 1/(1 + e^(SHARP*F - SHARP)); F clamped at
    1.085 so e^x <= 8.2e36 (recip_approx_fast safe; sigma there ~ 0).
  - vis_p = K0 * exp(-TAU * cum_occ_p) with K0 = exp(-TAU*occ0) folded into
    the HG / dt0 / dt10 weights (depth is linear in vis).
"""

from contextlib import ExitStack

import numpy as np

import concourse.bass as bass
import concourse.bacc as bacc
import concourse.mybir as mybir
from concourse import tile
from concourse.bass_utils import run_bass_kernel_spmd

F32 = mybir.dt.float32
BF16 = mybir.dt.bfloat16
AF = mybir.ActivationFunctionType
OP = mybir.AluOpType

# renderer constants (match the nn.Module init)
HS, WS = 360, 640
NEAR, FAR = 0.0, 1.5
NS = 10
SHARP = 1000.0
TAU = 100.0
N_SQ = 8
EPS = 1e-6
FCLAMP = 0.0815838  # ln(1.085): F clamp in log domain

N_CORES = 8
NRL = HS // N_CORES       # 45 local rows per core
NJ = WS // 128            # 5 x-blocks
NCOL = NRL * NJ           # 225 columns per core
P = 128


def _f(x):
    return float(np.float32(x))


def _host_consts(sq_poses, sq_params, rays_o, t):
    """Per-SQ scalars, computed in float64 from the f32 inputs."""
    sq_poses = np.asarray(sq_poses, np.float64)
    sq_params = np.asarray(sq_params, np.float64)
    rays_o = np.asarray(rays_o, np.float64)
    t = np.asarray(t, np.float64)

    consts = []
    for k in range(N_SQ):
        R = sq_poses[k, :3, :3]
        p = sq_poses[k, :3, 3]
        s = sq_params[k, 0:3]
        e1 = sq_params[k, 3]
        e2 = sq_params[k, 4]

        M1 = R.T / s[:, None]            # u = M1 @ d = (R^T d)/s
        tc = (R.T @ (rays_o - p)) / s
        rp = R.T @ p                      # loc(near) = -rp
        rps = rp / s
        c1 = 2.0 / e2
        c2 = e2 / e1
        c3 = 2.0 / e1

        # near-point occupancy (constant per SQ)
        Xn = np.abs(-rp) / s + EPS
        fN = (Xn[0] ** c1 + Xn[1] ** c1) ** c2 + Xn[2] ** c3
        Fn = fN ** e1
        with np.errstate(over="ignore"):
            occ0 = 1.0 / (1.0 + np.exp(-SHARP * (1.0 - Fn)))
        vis0 = np.exp(-TAU * occ0)       # == K0

        consts.append(dict(
            M1=M1, tc=tc, rp=rp, rps=rps, s=s,
            c1=c1, c2=c2, c3=c3, e1=e1,
            occ0=occ0, vis0=vis0,
        ))

    # segment weights from t (shared across SQs)
    dt_abs = np.abs(np.diff(t))          # |t_i - t_{i-1}|, i=1..9
    beta = np.zeros(11)                  # weight of v_s (s=1..10) in inner sum
    for i in range(1, NS):               # inner gaps i=1..9 use v_i, v_{i+1}
        beta[i] += 0.5 * dt_abs[i - 1]
        beta[i + 1] += 0.5 * dt_abs[i - 1]
    return consts, t, beta


def _host_rects(consts, rays_d):
    """Per-SQ (lr0, nr, j0, nj) bounding rect, identical across cores.

    h(d) = (b.d)^2 - (C-3) d^T A d is degree-2 homogeneous in d, so the
    coarse-subgrid sign test needs no ray normalization.  Conservative by a
    9px margin (>> 3px grid step; min blob diameter is ~40px for any SQ with
    C comfortably > 3).  Rows rounded to multiples of 8 so that every core's
    local-row range is the same [lr0, lr0+nr).
    """
    d = np.asarray(rays_d, np.float64)
    ys = np.arange(0, HS, 2)
    xs = np.arange(0, WS, 2)
    sub = d[np.ix_(ys, xs)]
    rects = []
    for cc in consts:
        M1, tcv = cc["M1"], cc["tc"]
        C = float((tcv ** 2).sum())
        if C <= 3.5:                      # near/inside bounding sphere: dense
            rects.append((0, NRL, 0, NJ))
            continue
        A = M1.T @ M1
        b = M1.T @ tcv
        hq = (sub @ b) ** 2 - (C - 3.0) * np.einsum("yxi,ij,yxj->yx", sub, A, sub)
        hit = hq > 0
        if not hit.any():
            rects.append(None)
            continue
        ryy, rxx = np.where(hit)
        r0 = max(0, int(ys[ryy.min()]) - 3)
        r1 = min(HS - 1, int(ys[ryy.max()]) + 3)
        x0 = max(0, int(xs[rxx.min()]) - 3)
        x1 = min(WS - 1, int(xs[rxx.max()]) + 3)
        r0 = (r0 // 8) * 8
        r1 = min(HS, ((r1 + 8) // 8) * 8) - 1
        lr0, nr = r0 // 8, (r1 - r0 + 1) // 8
        j0, j1 = x0 // 128, x1 // 128
        rects.append((lr0, nr, j0, j1 - j0 + 1))
    return rects


def build_program(consts, t, beta, rects, act_loads=True):
    """One SPMD program; input rdin [128,3,45,5], output depth [128,45,5]."""
    nc = bacc.Bacc("TRN2", target_bir_lowering=False, debug=False,
                   enable_asserts=False, num_devices=N_CORES)

    rd_dram = nc.dram_tensor("rdin", [P, 3, NRL, NJ], F32, kind="ExternalInput")
    out_dram = nc.dram_tensor("depth", [P, NRL, NJ], F32, kind="ExternalOutput")

    # const APs for activation biases (only 0.0/1.0 are pre-registered)
    def reg_const(v):
        v = _f(v)
        if (F32, v) not in nc.const_aps.aps:
            th = nc.alloc_sbuf_tensor(f"constap{len(nc.const_aps.aps)}", [128, 1], F32)
            nc.gpsimd.memset(th.ap(), v)
            nc.const_aps.aps[(F32, v)] = th.ap()

    reg_const(EPS)
    reg_const(-SHARP)
    nc.all_engine_barrier()

    live = [k for k in range(N_SQ) if rects[k] is not None]
    X = {k: rects[k][1] * rects[k][3] for k in live}
    off = {}
    o = 0
    for k in live:
        off[k] = o
        o += X[k]
    STOT = o
    # chunks of 2 SQs for software pipelining across engine phases
    chunks = [live[i:i + 2] for i in range(0, len(live), 2)]

    with tile.TileContext(nc) as tc, ExitStack() as es:
        V = nc.vector
        S = nc.scalar
        G = nc.gpsimd
        persist = es.enter_context(tc.tile_pool(name="persist", bufs=1))
        pool = es.enter_context(tc.tile_pool(name="scratch", bufs=2))

        # ---- shared loads & per-core shared prep ----
        rd = persist.tile([P, 3, NRL, NJ], F32, name="rd")
        nc.sync.dma_start(rd[:, :, :, :], rd_dram.ap())

        rdsq = persist.tile([P, 3, NRL, NJ], F32, name="rdsq")
        S.activation(rdsq[:, :, :, :], rd[:, :, :, :], AF.Square)
        nd = persist.tile([P, NRL, NJ], F32, name="nd")
        V.scalar_tensor_tensor(nd[:, :, :], rdsq[:, 0, :, :], 1.0,
                               rdsq[:, 1, :, :], OP.mult, OP.add)
        V.scalar_tensor_tensor(nd[:, :, :], rdsq[:, 2, :, :], 1.0,
                               nd[:, :, :], OP.mult, OP.add)
        S.activation(nd[:, :, :], nd[:, :, :], AF.Ln)
        S.activation(nd[:, :, :], nd[:, :, :], AF.Exp, scale=0.5)

        dmin = persist.tile([P, NRL, NJ], F32, name="dmin")
        G.memset(dmin[:, :, :], FAR)

        # beta weight planes (shared across SQs; value constant along cols)
        betaE = persist.tile([P, 5, STOT], BF16, name="betaE")
        betaO = persist.tile([P, 5, STOT], BF16, name="betaO")
        for i in range(5):
            G.memset(betaE[:, i, :], _f(beta[2 * i + 1]))   # v_1,3,5,7,9
            G.memset(betaO[:, i, :], _f(beta[2 * i + 2]))   # v_2,4,6,8,10

        # packed per-pixel planes (flat column axis over all live SQ rects)
        UB = persist.tile([P, 3, STOT], F32, name="UB")
        D1B = persist.tile([P, STOT], F32, name="D1B")
        RQB = persist.tile([P, STOT], F32, name="RQB")
        HCL = persist.tile([P, STOT], F32, name="HCL")     # hclsq -> hcl
        CENB = persist.tile([P, 3, STOT], BF16, name="CENB")
        HTDB = persist.tile([P, 3, STOT], BF16, name="HTDB")
        HGB = persist.tile([P, STOT], F32, name="HGB")     # nd*hcl*K0
        DTB = persist.tile([P, 2, STOT], F32, name="DTB")  # 0.5*K0*|tau|*nd
        PLB = persist.tile([P, NS + 1, 3, STOT], BF16, name="PLB")
        LNB = persist.tile([P, NS + 1, 3, STOT], F32, name="LNB")
        EB = persist.tile([P, NS + 1, STOT], F32, name="EB")
        OCC = persist.tile([P, NS + 1, STOT], F32, name="OCC")
        PO = persist.tile([P, 5, STOT], F32, name="PO")
        CUE = persist.tile([P, 6, STOT], F32, name="CUE")
        VO = persist.tile([P, 5, STOT], BF16, name="VO")
        VE = persist.tile([P, 6, STOT], BF16, name="VE")
        WV = persist.tile([P, 10, STOT], BF16, name="WV")
        ACC = persist.tile([P, STOT], F32, name="ACC")

        def xsl(k):
            return slice(off[k], off[k] + X[k])

        def csl(c):
            ks = chunks[c]
            return slice(off[ks[0]], off[ks[-1]] + X[ks[-1]])

        def r4(ap2, k):
            nj = rects[k][3]
            return ap2.rearrange("p (a b) -> p a b", b=nj)

        # ---------------- stage functions (per chunk) ----------------------
        def st_geom_a(c):
            for k in chunks[c]:
                cc = consts[k]
                lr0, nr, j0, nj = rects[k]
                Xk, xk = X[k], xsl(k)
                M1, tcv = cc["M1"], cc["tc"]
                rv = [rd[:, jj, lr0:lr0 + nr, j0:j0 + nj] for jj in range(3)]
                for j in range(3):
                    uo = r4(UB[:, j, xk], k)
                    V.tensor_scalar(uo, rv[0], _f(M1[j, 0]), None, OP.mult)
                    V.scalar_tensor_tensor(uo, rv[1], _f(M1[j, 1]), uo, OP.mult, OP.add)
                    V.scalar_tensor_tensor(uo, rv[2], _f(M1[j, 2]), uo, OP.mult, OP.add)
                uk = UB[:, :, xk]
                usq = pool.tile([P, 3, Xk], F32, tag="usq")
                V.scalar_tensor_tensor(usq[:, :, :], uk, 1.0, uk, OP.mult, OP.mult)
                nu2 = pool.tile([P, Xk], F32, tag="nu2")
                V.scalar_tensor_tensor(nu2[:], usq[:, 0, :], 1.0, usq[:, 1, :], OP.mult, OP.add)
                V.scalar_tensor_tensor(nu2[:], usq[:, 2, :], 1.0, nu2[:], OP.mult, OP.add)
                rqs = pool.tile([P, Xk], F32, tag="rqs")
                V.reciprocal_approx_accurate(RQB[:, xk], nu2[:], rqs[:])
                V.tensor_scalar(D1B[:, xk], UB[:, 0, xk], _f(-tcv[0]), None, OP.mult)
                V.scalar_tensor_tensor(D1B[:, xk], UB[:, 1, xk], _f(-tcv[1]),
                                       D1B[:, xk], OP.mult, OP.add)
                V.scalar_tensor_tensor(D1B[:, xk], UB[:, 2, xk], _f(-tcv[2]),
                                       D1B[:, xk], OP.mult, OP.add)
                proj = pool.tile([P, Xk], F32, tag="proj")
                V.scalar_tensor_tensor(proj[:], D1B[:, xk], 0.0, RQB[:, xk],
                                       OP.max, OP.mult)
                cen = pool.tile([P, 3, Xk], F32, tag="cen")
                projB = proj[:].unsqueeze(1).broadcast_to((P, 3, Xk))
                V.scalar_tensor_tensor(cen[:, :, :], projB, 1.0, uk, OP.mult, OP.mult)
                for j in range(3):
                    V.tensor_scalar(cen[:, j, :], cen[:, j, :], _f(tcv[j]), None, OP.add)
                V.tensor_copy(CENB[:, :, xk], cen[:, :, :])
                csq = pool.tile([P, 3, Xk], F32, tag="usq")
                V.scalar_tensor_tensor(csq[:, :, :], cen[:, :, :], 1.0,
                                       cen[:, :, :], OP.mult, OP.mult)
                m3 = pool.tile([P, Xk], F32, tag="m3")
                V.scalar_tensor_tensor(m3[:], csq[:, 0, :], 1.0, csq[:, 1, :], OP.mult, OP.add)
                V.scalar_tensor_tensor(m3[:], csq[:, 2, :], 1.0, m3[:], OP.mult, OP.add)
                V.tensor_scalar(m3[:], m3[:], -1.0, 3.0, OP.mult, OP.add)
                V.scalar_tensor_tensor(HCL[:, xk], m3[:], 1e-12, RQB[:, xk],
                                       OP.max, OP.mult)

        def st_hcl(c):
            r = csl(c)
            S.activation(HCL[:, r], HCL[:, r], AF.Ln)
            S.activation(HCL[:, r], HCL[:, r], AF.Exp, scale=0.5)

        def st_geom_b(c):
            for k in chunks[c]:
                cc = consts[k]
                Xk, xk = X[k], xsl(k)
                lr0, nr, j0, nj = rects[k]
                K0 = cc["vis0"]
                rps = cc["rps"]
                hclv = HCL[:, xk]
                hclB = hclv.unsqueeze(1).broadcast_to((P, 3, Xk))
                V.scalar_tensor_tensor(HTDB[:, :, xk], hclB, 1.0, UB[:, :, xk],
                                       OP.mult, OP.mult)
                ndv = nd[:, lr0:lr0 + nr, j0:j0 + nj]
                V.scalar_tensor_tensor(r4(HGB[:, xk], k), r4(hclv, k), _f(K0),
                                       ndv, OP.mult, OP.mult)
                base = pool.tile([P, Xk], F32, tag="base")
                V.scalar_tensor_tensor(base[:], D1B[:, xk], 1.0, RQB[:, xk],
                                       OP.mult, OP.mult)
                tau = pool.tile([P, 2, Xk], F32, tag="tau")
                V.scalar_tensor_tensor(tau[:, 0, :], hclv, _f(t[0]), base[:],
                                       OP.mult, OP.add)
                V.scalar_tensor_tensor(tau[:, 1, :], hclv, _f(t[NS - 1]), base[:],
                                       OP.mult, OP.add)
                V.tensor_scalar(tau[:, 1, :], tau[:, 1, :], -1.0, FAR, OP.mult, OP.add)
                V.scalar_tensor_tensor(tau[:, :, :], tau[:, :, :], -1.0,
                                       tau[:, :, :], OP.mult, OP.max)
                V.scalar_tensor_tensor(r4(DTB[:, 0, xk], k), r4(tau[:, 0, :], k),
                                       _f(0.5 * K0), ndv, OP.mult, OP.mult)
                V.scalar_tensor_tensor(r4(DTB[:, 1, xk], k), r4(tau[:, 1, :], k),
                                       _f(0.5 * K0), ndv, OP.mult, OP.mult)
                for j in range(3):
                    V.tensor_scalar(PLB[:, NS, j, xk], UB[:, j, xk], FAR,
                                    _f(-rps[j]), OP.mult, OP.add)

        def st_pl(c):
            r = csl(c)
            for si in range(NS):
                V.scalar_tensor_tensor(PLB[:, si, :, r], HTDB[:, :, r], _f(t[si]),
                                       CENB[:, :, r], OP.mult, OP.add)
            V.scalar_tensor_tensor(PLB[:, :, :, r], PLB[:, :, :, r], -1.0,
                                   PLB[:, :, :, r], OP.mult, OP.max)

        def st_chain1(c):
            r = csl(c)
            S.activation(LNB[:, :, :, r], PLB[:, :, :, r], AF.Ln, bias=_f(EPS))
            for k in chunks[c]:
                cc = consts[k]
                xk = xsl(k)
                S.activation(PLB[:, :, 0:2, xk], LNB[:, :, 0:2, xk], AF.Exp,
                             scale=_f(cc["c1"]))
                S.activation(PLB[:, :, 2, xk], LNB[:, :, 2, xk], AF.Exp,
                             scale=_f(cc["c3"]))

        def st_chain2(c):
            r = csl(c)
            V.scalar_tensor_tensor(PLB[:, :, 0, r], PLB[:, :, 1, r], 1.0,
                                   PLB[:, :, 0, r], OP.mult, OP.add)
            S.activation(LNB[:, :, 0, r], PLB[:, :, 0, r], AF.Ln)
            for k in chunks[c]:
                xk = xsl(k)
                S.activation(PLB[:, :, 0, xk], LNB[:, :, 0, xk], AF.Exp,
                             scale=_f(consts[k]["c2"]))
            V.scalar_tensor_tensor(PLB[:, :, 0, r], PLB[:, :, 2, r], 1.0,
                                   PLB[:, :, 0, r], OP.mult, OP.add)
            S.activation(LNB[:, :, 0, r], PLB[:, :, 0, r], AF.Ln)
            for k in chunks[c]:
                xk = xsl(k)
                V.tensor_scalar(LNB[:, :, 0, xk], LNB[:, :, 0, xk],
                                _f(consts[k]["e1"]), FCLAMP, OP.mult, OP.min)
            S.activation(EB[:, :, r], LNB[:, :, 0, r], AF.Exp)
            S.activation(EB[:, :, r], EB[:, :, r], AF.Exp,
                         scale=SHARP, bias=-SHARP)

        def st_occ(c):
            r = csl(c)
            V.tensor_scalar(EB[:, :, r], EB[:, :, r], 1.0, None, OP.add)
            V.reciprocal_approx_fast(OCC[:, :, r], EB[:, :, r])
            V.scalar_tensor_tensor(PO[:, :, r], OCC[:, 0:NS:2, r], 1.0,
                                   OCC[:, 1:NS + 1:2, r], OP.mult, OP.add)
            for i in range(1, 5):
                V.scalar_tensor_tensor(PO[:, i, r], PO[:, i - 1, r], 1.0,
                                       PO[:, i, r], OP.mult, OP.add)
            V.tensor_copy(CUE[:, 0, r], OCC[:, 0, r])
            V.scalar_tensor_tensor(CUE[:, 1:6, r], PO[:, :, r], 1.0,
                                   OCC[:, 2:NS + 1:2, r], OP.mult, OP.add)
            S.activation(VO[:, :, r], PO[:, :, r], AF.Exp, scale=-TAU)
            S.activation(VE[:, :, r], CUE[:, :, r], AF.Exp, scale=-TAU)

        def st_fin(c):
            r = csl(c)
            V.scalar_tensor_tensor(WV[:, 0:5, r], VE[:, 0:5, r], 1.0,
                                   betaE[:, :, r], OP.mult, OP.mult)
            V.scalar_tensor_tensor(WV[:, 5:10, r], VO[:, :, r], 1.0,
                                   betaO[:, :, r], OP.mult, OP.mult)
            V.scalar_tensor_tensor(WV[:, 0:5, r], WV[:, 0:5, r], 1.0,
                                   WV[:, 5:10, r], OP.mult, OP.add)
            V.scalar_tensor_tensor(WV[:, 0:2, r], WV[:, 0:2, r], 1.0,
                                   WV[:, 2:4, r], OP.mult, OP.add)
            V.scalar_tensor_tensor(ACC[:, r], WV[:, 0, r], 1.0, WV[:, 1, r],
                                   OP.mult, OP.add)
            V.scalar_tensor_tensor(ACC[:, r], WV[:, 4, r], 1.0, ACC[:, r],
                                   OP.mult, OP.add)
            V.scalar_tensor_tensor(ACC[:, r], ACC[:, r], 1.0, HGB[:, r],
                                   OP.mult, OP.mult)
            rc = r.stop - r.start
            b1 = pool.tile([P, rc], F32, tag="b1")
            V.scalar_tensor_tensor(b1[:], VE[:, 0, r], 1.0, DTB[:, 0, r],
                                   OP.add, OP.mult)
            V.scalar_tensor_tensor(ACC[:, r], b1[:], 1.0, ACC[:, r],
                                   OP.mult, OP.add)
            b2 = pool.tile([P, rc], F32, tag="b2")
            V.scalar_tensor_tensor(b2[:], VO[:, 4, r], 1.0, VE[:, 5, r],
                                   OP.mult, OP.add)
            V.scalar_tensor_tensor(b2[:], b2[:], 1.0, DTB[:, 1, r],
                                   OP.mult, OP.mult)
            V.scalar_tensor_tensor(ACC[:, r], b2[:], 1.0, ACC[:, r],
                                   OP.mult, OP.add)
            for k in chunks[c]:
                lr0, nr, j0, nj = rects[k]
                dv = dmin[:, lr0:lr0 + nr, j0:j0 + nj]
                V.scalar_tensor_tensor(dv, r4(ACC[:, xsl(k)], k), 1.0, dv,
                                       OP.mult, OP.min)

        stages = [st_geom_a, st_hcl, st_geom_b, st_pl, st_chain1,
                  st_chain2, st_occ, st_fin]
        NSTAGE = len(stages)
        C = len(chunks)
        for tick in range(C + NSTAGE - 1):
            for s_i in reversed(range(NSTAGE)):
                cidx = tick - s_i
                if 0 <= cidx < C:
                    stages[s_i](cidx)

        nc.sync.dma_start(out_dram.ap(), dmin[:, :, :])

    nc.compile()
    return nc


def _shard_rays(rays_d):
    """-> per-core arrays [128, 3, 45, 5]; core c owns rows 8*lr+c."""
    rd = np.asarray(rays_d, np.float32)
    out = []
    for c in range(N_CORES):
        sub = rd[c::N_CORES]                         # (45, 640, 3)
        arr = sub.reshape(NRL, NJ, 128, 3).transpose(2, 3, 0, 1)
        out.append(np.ascontiguousarray(arr))        # (128, 3, 45, 5)
    return out


def _unshard(outs):
    """outs: list of 8 arrays [128, 45, 5] -> (360, 640)."""
    full = np.empty((HS, WS), np.float32)
    for c in range(N_CORES):
        full[c::N_CORES] = outs[c].transpose(1, 2, 0).reshape(NRL, WS)
    return full


def kernel(sq_poses, sq_params, rays_d, rays_o, t, **run_kwargs):
    consts, tv, beta = _host_consts(sq_poses, sq_params, rays_o, t)
    rects = _host_rects(consts, rays_d)
    nc = build_program(consts, tv, beta, rects)
    planes = _shard_rays(rays_d)
    in_maps = [{"rdin": planes[c]} for c in range(N_CORES)]
    res = run_bass_kernel_spmd(nc, in_maps, core_ids=list(range(N_CORES)), **run_kwargs)
    outs = [res.results[c]["depth"] for c in range(N_CORES)]
    out = _unshard(outs).astype(np.float32)
    kernel.last_result = res
    return out


kernel.last_result = None


# revision 13
# speedup vs baseline: 1.2099x; 1.2099x over previous
"""Trainium2 Bass kernel for nn_DepthRenderer (superquadric depth renderer), v2.

Sharding: rows round-robin over 8 cores (core c owns image rows r = 8*lr+c,
lr=0..44).  Per-core layout [128 lanes, 45 lrows, 5 xblocks]; lane = x%128,
xblock = x//128.  Each core renders all 8 SQs (constants baked as immediates
into one SPMD program) and min-accumulates depth on device; host concatenates.

Sparsity: per-SQ bounding rects on a coarse host grid (see _host_rects).
Rect columns of all live SQs are packed into one flat column axis of size
STOT; per-SQ geometry writes packed planes, the sample/occupancy pipeline
runs batched per chunk of SQs, software-pipelined across chunks so the
Vector (DVE) and Scalar (ACT) engines overlap.

v2 speedups over the baseline:
  - bf16 on the sample-dim DVE ops (PL build, |PL|, S/f adds, weights):
    tensor_scalar/scalar_tensor_tensor run in 4x DVE mode on packed bf16.
  - fp32 tensor_tensor ops replaced by scalar_tensor_tensor with a dummy
    x1.0 scalar stage: InstTensorScalarPtr supports the 2x_2p DVE mode in
    fp32 (plain InstTensorTensor does not).
  - ACT Abs eliminated (|PL| via one DVE STT max(-x,x)); ACT Identity(+1)
    eliminated (occ denominator built with a DVE tensor_scalar add).
  - per-SQ constant folds: K0 = exp(-TAU*occ0) folded into HG and the
    boundary dt terms (so the occ0 cumsum add disappears); the F clamp is
    applied in log domain (one TS: min(e1*lnf, ln 1.085)).
  - batched ACT chain per SQ-chunk (ln/lnS/lnf/F/E shared-scale ops cover a
    whole chunk in one instruction).

Math notes (exact rewrites of the reference, up to fp rounding):
  - a == sizes  =>  X = |loc|/a + eps = |pts_loc| + eps  (sizes cancel)
  - samples lie on the ray: tau_s = d1/nu2 + hcl*t_s, so boundary segment
    lengths need no norms: dt0 = |tau_0|*||d||, dt10 = |1.5 - tau_9|*||d||
  - occ = sigmoid(SHARP*(1-F)) = 1/(1 + e^(SHARP*F - SHARP)); F clamped at
    1.085 so e^x <= 8.2e36 (recip_approx_fast safe; sigma there ~ 0).
  - vis_p = K0 * exp(-TAU * cum_occ_p) with K0 = exp(-TAU*occ0) folded into
    the HG / dt0 / dt10 weights (depth is linear in vis).
"""

from contextlib import ExitStack

import numpy as np

import concourse.bass as bass
import concourse.bacc as bacc
import concourse.mybir as mybir
from concourse import tile
from concourse.bass_utils import run_bass_kernel_spmd

F32 = mybir.dt.float32
BF16 = mybir.dt.bfloat16
AF = mybir.ActivationFunctionType
OP = mybir.AluOpType

# renderer constants (match the nn.Module init)
HS, WS = 360, 640
NEAR, FAR = 0.0, 1.5
NS = 10
SHARP = 1000.0
TAU = 100.0
N_SQ = 8
EPS = 1e-6
FCLAMP = 0.0815838  # ln(1.085): F clamp in log domain

N_CORES = 8
NRL = HS // N_CORES       # 45 local rows per core
NJ = WS // 128            # 5 x-blocks
NCOL = NRL * NJ           # 225 columns per core
P = 128


def _f(x):
    return float(np.float32(x))


def _host_consts(sq_poses, sq_params, rays_o, t):
    """Per-SQ scalars, computed in float64 from the f32 inputs."""
    sq_poses = np.asarray(sq_poses, np.float64)
    sq_params = np.asarray(sq_params, np.float64)
    rays_o = np.asarray(rays_o, np.float64)
    t = np.asarray(t, np.float64)

    consts = []
    for k in range(N_SQ):
        R = sq_poses[k, :3, :3]
        p = sq_poses[k, :3, 3]
        s = sq_params[k, 0:3]
        e1 = sq_params[k, 3]
        e2 = sq_params[k, 4]

        M1 = R.T / s[:, None]            # u = M1 @ d = (R^T d)/s
        tc = (R.T @ (rays_o - p)) / s
        rp = R.T @ p                      # loc(near) = -rp
        rps = rp / s
        c1 = 2.0 / e2
        c2 = e2 / e1
        c3 = 2.0 / e1

        # near-point occupancy (constant per SQ)
        Xn = np.abs(-rp) / s + EPS
        fN = (Xn[0] ** c1 + Xn[1] ** c1) ** c2 + Xn[2] ** c3
        Fn = fN ** e1
        with np.errstate(over="ignore"):
            occ0 = 1.0 / (1.0 + np.exp(-SHARP * (1.0 - Fn)))
        vis0 = np.exp(-TAU * occ0)       # == K0

        consts.append(dict(
            M1=M1, tc=tc, rp=rp, rps=rps, s=s,
            c1=c1, c2=c2, c3=c3, e1=e1,
            occ0=occ0, vis0=vis0,
        ))

    # segment weights from t (shared across SQs)
    dt_abs = np.abs(np.diff(t))          # |t_i - t_{i-1}|, i=1..9
    beta = np.zeros(11)                  # weight of v_s (s=1..10) in inner sum
    for i in range(1, NS):               # inner gaps i=1..9 use v_i, v_{i+1}
        beta[i] += 0.5 * dt_abs[i - 1]
        beta[i + 1] += 0.5 * dt_abs[i - 1]
    return consts, t, beta


def _host_rects(consts, rays_d):
    """Per-SQ (lr0, nr, j0, nj) bounding rect, identical across cores.

    h(d) = (b.d)^2 - (C-3) d^T A d is degree-2 homogeneous in d, so the
    coarse-subgrid sign test needs no ray normalization.  Conservative by a
    9px margin (>> 3px grid step; min blob diameter is ~40px for any SQ with
    C comfortably > 3).  Rows rounded to multiples of 8 so that every core's
    local-row range is the same [lr0, lr0+nr).
    """
    d = np.asarray(rays_d, np.float64)
    ys = np.arange(0, HS, 2)
    xs = np.arange(0, WS, 2)
    sub = d[np.ix_(ys, xs)]
    rects = []
    for cc in consts:
        M1, tcv = cc["M1"], cc["tc"]
        C = float((tcv ** 2).sum())
        if C <= 3.5:                      # near/inside bounding sphere: dense
            rects.append((0, NRL, 0, NJ))
            continue
        A = M1.T @ M1
        b = M1.T @ tcv
        hq = (sub @ b) ** 2 - (C - 3.0) * np.einsum("yxi,ij,yxj->yx", sub, A, sub)
        hit = hq > 0
        if not hit.any():
            rects.append(None)
            continue
        ryy, rxx = np.where(hit)
        r0 = max(0, int(ys[ryy.min()]) - 3)
        r1 = min(HS - 1, int(ys[ryy.max()]) + 3)
        x0 = max(0, int(xs[rxx.min()]) - 3)
        x1 = min(WS - 1, int(xs[rxx.max()]) + 3)
        r0 = (r0 // 8) * 8
        r1 = min(HS, ((r1 + 8) // 8) * 8) - 1
        lr0, nr = r0 // 8, (r1 - r0 + 1) // 8
        j0, j1 = x0 // 128, x1 // 128
        rects.append((lr0, nr, j0, j1 - j0 + 1))
    return rects


def build_program(consts, t, beta, rects, act_loads=True):
    """One SPMD program; input rdin [128,3,45,5], output depth [128,45,5]."""
    nc = bacc.Bacc("TRN2", target_bir_lowering=False, debug=False,
                   enable_asserts=False, num_devices=N_CORES)

    rd_dram = nc.dram_tensor("rdin", [P, 3, NRL, NJ], F32, kind="ExternalInput")
    out_dram = nc.dram_tensor("depth", [P, NRL, NJ], F32, kind="ExternalOutput")

    # const APs for activation biases (only 0.0/1.0 are pre-registered)
    def reg_const(v):
        v = _f(v)
        if (F32, v) not in nc.const_aps.aps:
            th = nc.alloc_sbuf_tensor(f"constap{len(nc.const_aps.aps)}", [128, 1], F32)
            nc.gpsimd.memset(th.ap(), v)
            nc.const_aps.aps[(F32, v)] = th.ap()

    reg_const(EPS)
    reg_const(-SHARP)
    nc.all_engine_barrier()

    live = [k for k in range(N_SQ) if rects[k] is not None]
    X = {k: rects[k][1] * rects[k][3] for k in live}
    off = {}
    o = 0
    for k in live:
        off[k] = o
        o += X[k]
    STOT = o
    # chunks of 2 SQs for software pipelining across engine phases
    chunks = [live[i:i + 2] for i in range(0, len(live), 2)]

    with tile.TileContext(nc) as tc, ExitStack() as es:
        V = nc.vector
        S = nc.scalar
        G = nc.gpsimd
        persist = es.enter_context(tc.tile_pool(name="persist", bufs=1))
        pool = es.enter_context(tc.tile_pool(name="scratch", bufs=2))

        # ---- shared loads & per-core shared prep ----
        rd = persist.tile([P, 3, NRL, NJ], F32, name="rd")
        nc.sync.dma_start(rd[:, :, :, :], rd_dram.ap())

        rdsq = persist.tile([P, 3, NRL, NJ], F32, name="rdsq")
        S.activation(rdsq[:, :, :, :], rd[:, :, :, :], AF.Square)
        nd = persist.tile([P, NRL, NJ], F32, name="nd")
        V.scalar_tensor_tensor(nd[:, :, :], rdsq[:, 0, :, :], 1.0,
                               rdsq[:, 1, :, :], OP.mult, OP.add)
        V.scalar_tensor_tensor(nd[:, :, :], rdsq[:, 2, :, :], 1.0,
                               nd[:, :, :], OP.mult, OP.add)
        S.activation(nd[:, :, :], nd[:, :, :], AF.Ln)
        S.activation(nd[:, :, :], nd[:, :, :], AF.Exp, scale=0.5)

        dmin = persist.tile([P, NRL, NJ], F32, name="dmin")
        G.memset(dmin[:, :, :], FAR)

        # beta weight planes (shared across SQs; value constant along cols)
        betaE = persist.tile([P, 5, STOT], BF16, name="betaE")
        betaO = persist.tile([P, 5, STOT], BF16, name="betaO")
        for i in range(5):
            G.memset(betaE[:, i, :], _f(beta[2 * i + 1]))   # v_1,3,5,7,9
            G.memset(betaO[:, i, :], _f(beta[2 * i + 2]))   # v_2,4,6,8,10

        # packed per-pixel planes (flat column axis over all live SQ rects)
        UB = persist.tile([P, 3, STOT], F32, name="UB")
        D1B = persist.tile([P, STOT], F32, name="D1B")
        RQB = persist.tile([P, STOT], F32, name="RQB")
        HCL = persist.tile([P, STOT], F32, name="HCL")     # hclsq -> hcl
        CENB = persist.tile([P, 3, STOT], BF16, name="CENB")
        HTDB = persist.tile([P, 3, STOT], BF16, name="HTDB")
        HGB = persist.tile([P, STOT], F32, name="HGB")     # nd*hcl*K0
        DTB = persist.tile([P, 2, STOT], F32, name="DTB")  # 0.5*K0*|tau|*nd
        PLB = persist.tile([P, NS + 1, 3, STOT], BF16, name="PLB")
        ABSB = persist.tile([P, NS + 1, 3, STOT], BF16, name="ABSB")
        LNB = persist.tile([P, NS + 1, 3, STOT], F32, name="LNB")
        EB = persist.tile([P, NS + 1, STOT], F32, name="EB")
        OCC = persist.tile([P, NS + 1, STOT], F32, name="OCC")
        PO = persist.tile([P, 5, STOT], F32, name="PO")
        CUE = persist.tile([P, 6, STOT], F32, name="CUE")
        VO = persist.tile([P, 5, STOT], BF16, name="VO")
        VE = persist.tile([P, 6, STOT], BF16, name="VE")
        WV = persist.tile([P, 10, STOT], BF16, name="WV")
        SS = persist.tile([P, 5, STOT], BF16, name="SS")
        ACC = persist.tile([P, STOT], F32, name="ACC")

        def xsl(k):
            return slice(off[k], off[k] + X[k])

        def csl(c):
            ks = chunks[c]
            return slice(off[ks[0]], off[ks[-1]] + X[ks[-1]])

        def r4(ap2, k):
            nj = rects[k][3]
            return ap2.rearrange("p (a b) -> p a b", b=nj)

        # ---------------- stage functions (per chunk) ----------------------
        def st_geom_a(c):
            for k in chunks[c]:
                cc = consts[k]
                lr0, nr, j0, nj = rects[k]
                Xk, xk = X[k], xsl(k)
                M1, tcv = cc["M1"], cc["tc"]
                C3 = _f(3.0 - float((cc["tc"] ** 2).sum()))
                rv = [rd[:, jj, lr0:lr0 + nr, j0:j0 + nj] for jj in range(3)]
                for j in range(3):
                    uo = r4(UB[:, j, xk], k)
                    V.tensor_scalar(uo, rv[0], _f(M1[j, 0]), None, OP.mult)
                    V.scalar_tensor_tensor(uo, rv[1], _f(M1[j, 1]), uo, OP.mult, OP.add)
                    V.scalar_tensor_tensor(uo, rv[2], _f(M1[j, 2]), uo, OP.mult, OP.add)
                uk = UB[:, :, xk]
                usq = pool.tile([P, 3, Xk], F32, tag="usq")
                V.scalar_tensor_tensor(usq[:, :, :], uk, 1.0, uk, OP.mult, OP.mult)
                nu2 = pool.tile([P, Xk], F32, tag="nu2")
                V.scalar_tensor_tensor(nu2[:], usq[:, 0, :], 1.0, usq[:, 1, :], OP.mult, OP.add)
                nu2b = pool.tile([P, Xk], F32, tag="nu2b")
                V.scalar_tensor_tensor(nu2b[:], usq[:, 2, :], 1.0, nu2[:], OP.mult, OP.add)
                rqs = pool.tile([P, Xk], F32, tag="rqs")
                V.reciprocal_approx_accurate(RQB[:, xk], nu2b[:], rqs[:])
                V.tensor_scalar(D1B[:, xk], UB[:, 0, xk], _f(-tcv[0]), None, OP.mult)
                V.scalar_tensor_tensor(D1B[:, xk], UB[:, 1, xk], _f(-tcv[1]),
                                       D1B[:, xk], OP.mult, OP.add)
                V.scalar_tensor_tensor(D1B[:, xk], UB[:, 2, xk], _f(-tcv[2]),
                                       D1B[:, xk], OP.mult, OP.add)
                proj = pool.tile([P, Xk], F32, tag="proj")
                V.scalar_tensor_tensor(proj[:], D1B[:, xk], 0.0, RQB[:, xk],
                                       OP.max, OP.mult)
                # cen (f32, single rounding into bf16 CENB for PL)
                projB = proj[:].unsqueeze(1).broadcast_to((P, 3, Xk))
                cen = pool.tile([P, 3, Xk], F32, tag="cen")
                V.scalar_tensor_tensor(cen[:, :, :], projB, 1.0, uk, OP.mult, OP.mult)
                for j in range(3):
                    V.tensor_scalar(cen[:, j, :], cen[:, j, :], _f(tcv[j]),
                                    None, OP.add)
                V.tensor_copy(CENB[:, :, xk], cen[:, :, :])
                # m3 = 3 - |cen|^2 == (3-C) + max(d1,0)^2 * rq  (exact)
                q = pool.tile([P, Xk], F32, tag="q")
                V.scalar_tensor_tensor(q[:], D1B[:, xk], 0.0, D1B[:, xk],
                                       OP.max, OP.mult)
                m3 = pool.tile([P, Xk], F32, tag="m3")
                V.scalar_tensor_tensor(m3[:], q[:], 1.0, RQB[:, xk], OP.mult, OP.mult)
                V.tensor_scalar(m3[:], m3[:], 1.0, C3, OP.mult, OP.add)
                V.scalar_tensor_tensor(HCL[:, xk], m3[:], 1e-12, RQB[:, xk],
                                       OP.max, OP.mult)

        def st_hcl(c):
            r = csl(c)
            S.activation(HCL[:, r], HCL[:, r], AF.Ln)
            S.activation(HCL[:, r], HCL[:, r], AF.Exp, scale=0.5)

        def st_geom_b(c):
            for k in chunks[c]:
                cc = consts[k]
                Xk, xk = X[k], xsl(k)
                lr0, nr, j0, nj = rects[k]
                K0 = cc["vis0"]
                rps = cc["rps"]
                hclv = HCL[:, xk]
                hclB = hclv.unsqueeze(1).broadcast_to((P, 3, Xk))
                V.scalar_tensor_tensor(HTDB[:, :, xk], hclB, 1.0, UB[:, :, xk],
                                       OP.mult, OP.mult)
                for j in range(3):
                    V.tensor_scalar(PLB[:, NS, j, xk], UB[:, j, xk], FAR,
                                    _f(-rps[j]), OP.mult, OP.add)
                # boundary-segment weights; base = d1*rq on the idle gpsimd
                ndv = nd[:, lr0:lr0 + nr, j0:j0 + nj]
                V.scalar_tensor_tensor(r4(HGB[:, xk], k), r4(hclv, k), _f(K0),
                                       ndv, OP.mult, OP.mult)
                base = pool.tile([P, Xk], F32, tag="base")
                V.scalar_tensor_tensor(base[:], D1B[:, xk], 1.0, RQB[:, xk],
                                       OP.mult, OP.mult)
                tau = pool.tile([P, 2, Xk], F32, tag="tau")
                V.scalar_tensor_tensor(tau[:, 0, :], hclv, _f(t[0]), base[:],
                                       OP.mult, OP.add)
                V.scalar_tensor_tensor(tau[:, 1, :], hclv, _f(t[NS - 1]), base[:],
                                       OP.mult, OP.add)
                V.tensor_scalar(tau[:, 1, :], tau[:, 1, :], -1.0, FAR, OP.mult, OP.add)
                tab = pool.tile([P, 2, Xk], F32, tag="tab")
                V.scalar_tensor_tensor(tab[:, :, :], tau[:, :, :], -1.0,
                                       tau[:, :, :], OP.mult, OP.max)
                V.scalar_tensor_tensor(r4(DTB[:, 0, xk], k), r4(tab[:, 0, :], k),
                                       _f(0.5 * K0), ndv, OP.mult, OP.mult)
                V.scalar_tensor_tensor(r4(DTB[:, 1, xk], k), r4(tab[:, 1, :], k),
                                       _f(0.5 * K0), ndv, OP.mult, OP.mult)

        def st_pl(c):
            r = csl(c)
            for si in range(NS):
                V.scalar_tensor_tensor(PLB[:, si, :, r], HTDB[:, :, r], _f(t[si]),
                                       CENB[:, :, r], OP.mult, OP.add)
            V.scalar_tensor_tensor(ABSB[:, :, :, r], PLB[:, :, :, r], -1.0,
                                   PLB[:, :, :, r], OP.mult, OP.max)

        def st_chain1(c):
            r = csl(c)
            S.activation(LNB[:, :, :, r], ABSB[:, :, :, r], AF.Ln, bias=_f(EPS))
            for k in chunks[c]:
                cc = consts[k]
                xk = xsl(k)
                S.activation(PLB[:, :, 0:2, xk], LNB[:, :, 0:2, xk], AF.Exp,
                             scale=_f(cc["c1"]))
                S.activation(PLB[:, :, 2, xk], LNB[:, :, 2, xk], AF.Exp,
                             scale=_f(cc["c3"]))

        def st_chain2(c):
            r = csl(c)
            V.scalar_tensor_tensor(ABSB[:, :, 0, r], PLB[:, :, 1, r], 1.0,
                                   PLB[:, :, 0, r], OP.mult, OP.add)
            S.activation(LNB[:, :, 0, r], ABSB[:, :, 0, r], AF.Ln)
            for k in chunks[c]:
                xk = xsl(k)
                S.activation(PLB[:, :, 0, xk], LNB[:, :, 0, xk], AF.Exp,
                             scale=_f(consts[k]["c2"]))
            V.scalar_tensor_tensor(ABSB[:, :, 0, r], PLB[:, :, 2, r], 1.0,
                                   PLB[:, :, 0, r], OP.mult, OP.add)
            S.activation(LNB[:, :, 0, r], ABSB[:, :, 0, r], AF.Ln)
            for k in chunks[c]:
                xk = xsl(k)
                V.tensor_scalar(LNB[:, :, 1, xk], LNB[:, :, 0, xk],
                                _f(consts[k]["e1"]), FCLAMP, OP.mult, OP.min)
            S.activation(EB[:, :, r], LNB[:, :, 1, r], AF.Exp)
            S.activation(EB[:, :, r], EB[:, :, r], AF.Exp,
                         scale=SHARP, bias=-SHARP)

        def st_occ(c):
            r = csl(c)
            V.tensor_scalar(OCC[:, :, r], EB[:, :, r], 1.0, None, OP.add)
            V.reciprocal_approx_fast(EB[:, :, r], OCC[:, :, r])
            V.scalar_tensor_tensor(PO[:, :, r], EB[:, 0:NS:2, r], 1.0,
                                   EB[:, 1:NS + 1:2, r], OP.mult, OP.add)
            for i in range(1, 5):
                V.scalar_tensor_tensor(PO[:, i, r], PO[:, i - 1, r], 1.0,
                                       PO[:, i, r], OP.mult, OP.add)
            V.tensor_copy(CUE[:, 0, r], EB[:, 0, r])
            V.scalar_tensor_tensor(CUE[:, 1:6, r], PO[:, :, r], 1.0,
                                   EB[:, 2:NS + 1:2, r], OP.mult, OP.add)
            S.activation(VO[:, :, r], PO[:, :, r], AF.Exp, scale=-TAU)
            S.activation(VE[:, :, r], CUE[:, :, r], AF.Exp, scale=-TAU)

        def st_fin(c):
            r = csl(c)
            V.scalar_tensor_tensor(WV[:, 0:5, r], VE[:, 0:5, r], 1.0,
                                   betaE[:, :, r], OP.mult, OP.mult)
            V.scalar_tensor_tensor(WV[:, 5:10, r], VO[:, :, r], 1.0,
                                   betaO[:, :, r], OP.mult, OP.mult)
            V.scalar_tensor_tensor(SS[:, :, r], WV[:, 0:5, r], 1.0,
                                   WV[:, 5:10, r], OP.mult, OP.add)
            V.scalar_tensor_tensor(SS[:, 0:2, r], SS[:, 0:2, r], 1.0,
                                   SS[:, 2:4, r], OP.mult, OP.add)
            V.scalar_tensor_tensor(ACC[:, r], SS[:, 0, r], 1.0, SS[:, 1, r],
                                   OP.mult, OP.add)
            V.scalar_tensor_tensor(ACC[:, r], SS[:, 4, r], 1.0, ACC[:, r],
                                   OP.mult, OP.add)
            V.scalar_tensor_tensor(ACC[:, r], ACC[:, r], 1.0, HGB[:, r],
                                   OP.mult, OP.mult)
            rc = r.stop - r.start
            b1 = pool.tile([P, rc], F32, tag="b1")
            V.scalar_tensor_tensor(b1[:], VE[:, 0, r], 1.0, DTB[:, 0, r],
                                   OP.add, OP.mult)
            V.scalar_tensor_tensor(ACC[:, r], b1[:], 1.0, ACC[:, r],
                                   OP.mult, OP.add)
            b2 = pool.tile([P, rc], F32, tag="b2")
            V.scalar_tensor_tensor(b2[:], VO[:, 4, r], 1.0, VE[:, 5, r],
                                   OP.mult, OP.add)
            V.scalar_tensor_tensor(b2[:], b2[:], 1.0, DTB[:, 1, r],
                                   OP.mult, OP.mult)
            V.scalar_tensor_tensor(ACC[:, r], b2[:], 1.0, ACC[:, r],
                                   OP.mult, OP.add)
            for k in chunks[c]:
                lr0, nr, j0, nj = rects[k]
                dv = dmin[:, lr0:lr0 + nr, j0:j0 + nj]
                V.scalar_tensor_tensor(dv, r4(ACC[:, xsl(k)], k), 1.0, dv,
                                       OP.mult, OP.min)

        stages = [st_geom_a, st_hcl, st_geom_b, st_pl, st_chain1,
                  st_chain2, st_occ, st_fin]
        NSTAGE = len(stages)
        C = len(chunks)
        for tick in range(C + NSTAGE - 1):
            for s_i in reversed(range(NSTAGE)):
                cidx = tick - s_i
                if 0 <= cidx < C:
                    stages[s_i](cidx)

        nc.sync.dma_start(out_dram.ap(), dmin[:, :, :])

    # Pre-place the single ACT table load (natural_log_exp covers Square/Ln/
    # Exp — everything this kernel uses) so bacc's fixpoint inserts no
    # further table switches.  Without this the auto-inserter thrashes
    # between exp_and_others and natural_log (27 loads = 34us measured).
    # (CoreSim can't handle the hand-inserted loads; act_loads=False skips.)
    if not act_loads:
        nc.compile()
        return nc
    from concourse.hw_specs import get_activation_tables
    names = list(get_activation_tables(nc.m.arch).keys())
    id_nle = names.index("natural_log_exp_and_others")

    def make_load(set_id):
        ins = mybir.InstLoadActFuncSet(
            name=nc.get_next_instruction_name(), act_func_set_id=set_id,
            ins=[], outs=[])
        ins.engine = nc.scalar.engine
        return ins

    for blk in nc.main_func.blocks:
        il = blk.instructions
        first_act = next((i for i, x in enumerate(il)
                          if isinstance(x, mybir.InstActivation)), None)
        if first_act is None:
            continue
        il.insert(first_act, make_load(id_nle))

    nc.compile()
    return nc


def _shard_rays(rays_d):
    """-> per-core arrays [128, 3, 45, 5]; core c owns rows 8*lr+c."""
    rd = np.asarray(rays_d, np.float32)
    out = []
    for c in range(N_CORES):
        sub = rd[c::N_CORES]                         # (45, 640, 3)
        arr = sub.reshape(NRL, NJ, 128, 3).transpose(2, 3, 0, 1)
        out.append(np.ascontiguousarray(arr))        # (128, 3, 45, 5)
    return out


def _unshard(outs):
    """outs: list of 8 arrays [128, 45, 5] -> (360, 640)."""
    full = np.empty((HS, WS), np.float32)
    for c in range(N_CORES):
        full[c::N_CORES] = outs[c].transpose(1, 2, 0).reshape(NRL, WS)
    return full


def kernel(sq_poses, sq_params, rays_d, rays_o, t, **run_kwargs):
    consts, tv, beta = _host_consts(sq_poses, sq_params, rays_o, t)
    rects = _host_rects(consts, rays_d)
    nc = build_program(consts, tv, beta, rects)
    planes = _shard_rays(rays_d)
    in_maps = [{"rdin": planes[c]} for c in range(N_CORES)]
    res = run_bass_kernel_spmd(nc, in_maps, core_ids=list(range(N_CORES)), **run_kwargs)
    outs = [res.results[c]["depth"] for c in range(N_CORES)]
    out = _unshard(outs).astype(np.float32)
    kernel.last_result = res
    return out


kernel.last_result = None
